# revision 1
# baseline (speedup 1.0000x reference)
"""Trainium2 Bass kernel for nn_LongThinNet (16-layer thin MLP, batch 2^20).

Strategy (pure data parallelism, batch sharded 8 ways):
  Per core 131072 rows as 128 j-slices of 1024 contiguous rows, grouped into
  11 tiles of 12 j-slices (last tile 8). Activations live feature-major:
  [121 partitions, 1024 batch] bf16 tiles, partition 10*band+feature for 12
  bands, partition 120 = constant-1 row used to carry biases into the final
  matmul. All matmuls bf16 (1 cycle/column on the PE) into fp32 PSUM.

  Elementwise Prelu(alpha=0.5) is the roofline for this net; it is load-
  balanced across ACT (single fused Prelu per [121,1024] tile) and DVE
  (tensor_scalar add-bias + scalar_tensor_tensor max(v, 0.5v)); GPSIMD is
  avoided (measured ~10x below its cost model and cannot access PSUM).

  Final layer transposes to batch-major via h-stationary matmuls into PSUM
  [128, 480] halves, copied to SBUF bf16 and DMAed out.
"""

import sys

sys.path.insert(0, "/opt/trn_rl_repo")

from contextlib import ExitStack

import numpy as np
import ml_dtypes

import concourse.bass as bass
import concourse.mybir as mybir
import concourse.tile as tile
from concourse.bass_utils import run_bass_kernel_spmd

F32 = mybir.dt.float32
BF16 = mybir.dt.bfloat16
AF = mybir.ActivationFunctionType
ALU = mybir.AluOpType
NPBF16 = ml_dtypes.bfloat16

NCORES = 8
BC = 131072          # rows per core
IN, HID = 40, 10
NMID = 14
JT = 1024            # rows per j-slice
NJ = BC // JT        # 128 j-slices
NT = 11              # tiles: 10 full (12 bands) + 1 with 8 bands
NBLK = 43            # x blocks of 3 j-slices (last holds 2, padded)

# Options per (layer, tile) act: one fused [121,1024] Prelu on ACT, or two
# [121,512] two-op Prelus on DVE. GPSIMD/Pool cannot touch PSUM and rejects
# tensor_tensor at codegen, so it only handles some L15 output copies.
# GPSIMD/Pool is excluded: despite the cost model's 0.6-efficiency estimate,
# real Q7 software tensor ops measured ~10x slower and it cannot touch PSUM.
ACT_OPTIONS = {
    "act": {"act": 1.038},
    "dve": {"dve": 2.38},
}
COPY_OPTIONS = {
    "act": {"act": 0.57},
    "dve": {"dve": 0.625},
}


def _assign_engines():
    """Greedy assignment minimizing the resulting max engine load."""
    opts = ACT_OPTIONS
    loads = {"act": 0.0, "dve": 0.0, "pool": 0.0}
    table = {}
    jobs = [("act", l, t) for l in range(NMID + 1) for t in range(NT)]
    jobs += [("copy", t, half) for t in range(NT) for half in range(2)]
    for job in jobs:
        options = opts if job[0] == "act" else COPY_OPTIONS
        best, best_val = None, None
        for name, deltas in options.items():
            val = max((loads[e] + d) for e, d in deltas.items())
            if best_val is None or val < best_val:
                best, best_val = name, val
        for e, d in options[best].items():
            loads[e] += d
        table[job] = best
    return table


ENGINE_TABLE = _assign_engines()


def _skip(name):
    return name in ("InstEventSemaphore", "InstAllEngineBarrier")


def _split_multi_waits(nc):
    """walrus codegen allows <=1 semaphore wait per instruction; hoist extras
    onto standalone InstEventSemaphore instructions inserted just before."""
    n_new = 0
    for f in nc.m.functions:
        for bb in f.blocks:
            out, changed = [], False
            for inst in bb.instructions:
                si = inst.sync_info
                if si is not None and len(si.on_wait) > 1 and not _skip(type(inst).__name__):
                    waits = list(si.on_wait)
                    for w in waits[:-1]:
                        n_new += 1
                        out.append(
                            mybir.InstEventSemaphore(
                                name=f"EVW-{n_new}-{inst.name}",
                                engine=inst.engine,
                                sync_info=mybir.SyncInfo(on_wait=[w], on_update=[]),
                            )
                        )
                    inst.sync_info = mybir.SyncInfo(
                        on_wait=[waits[-1]], on_update=list(si.on_update)
                    )
                    changed = True
                out.append(inst)
            if changed:
                try:
                    bb.instructions = out
                except Exception:
                    lst = bb.instructions
                    lst.clear()
                    lst.extend(out)
    return n_new


def _pack_weights(W_in, b_in, W_mid, b_mid, W_out, b_out):
    # wl0: 4 variants [120, 121]; variant k maps x block (3 j x 40 feats) to
    # bands 3k..3k+2 (partition 10q) of the tile psum.
    wl0 = np.zeros((120, 4 * 121), np.float32)
    for k in range(4):
        for g in range(3):
            q = 3 * k + g
            wl0[40 * g:40 * g + 40, 121 * k + 10 * q:121 * k + 10 * q + 10] = W_in.T

    # wmid: per layer [121, 121] block-diag 12x(10x10) + identity 1-row.
    wmid = np.zeros((121, NMID * 121), np.float32)
    for l in range(NMID):
        for q in range(12):
            wmid[10 * q:10 * q + 10, 121 * l + 10 * q:121 * l + 10 * q + 10] = W_mid[l].T
        wmid[120, 121 * l + 120] = 1.0

    # wl15: [121, 120] block-diag W_out.T + b_out via the 1-row.
    wl15 = np.zeros((121, 120), np.float32)
    for q in range(12):
        wl15[10 * q:10 * q + 10, 10 * q:10 * q + 10] = W_out.T
        wl15[120, 10 * q:10 * q + 10] = b_out

    # wbias: [121, 15] fp32; col 0 = L0 (b_in + 1.0 at p120), cols 1..14 mid.
    wbias = np.zeros((121, 15), np.float32)
    for q in range(12):
        wbias[10 * q:10 * q + 10, 0] = b_in
        for l in range(NMID):
            wbias[10 * q:10 * q + 10, 1 + l] = b_mid[l]
    wbias[120, 0] = 1.0

    return {
        "wl0": wl0.astype(NPBF16),
        "wmid": wmid.astype(NPBF16),
        "wl15": wl15.astype(NPBF16),
        "wbias": wbias,
    }


def _pack_x_core(xc):
    """[131072, 40] fp32 -> [43, 120, 1024] bf16 feature-major blocks:
    block k partition 40*g+f, col c = x[1024*(3k+g) + c, f]."""
    out = np.zeros((NBLK, 120, JT), NPBF16)
    a = xc[:42 * 3 * JT].reshape(42, 3, JT, IN).transpose(0, 1, 3, 2)
    out[:42] = a.reshape(42, 120, JT).astype(NPBF16)
    b = xc[42 * 3 * JT:].reshape(2, JT, IN).transpose(0, 2, 1)
    out[42, :80] = b.reshape(80, JT).astype(NPBF16)
    return out


def _build_nc(reps=1, split_waits=True):
    nc = bass.Bass("TRN2", target_bir_lowering=False, debug=False)

    x_d = nc.dram_tensor("x", [NBLK, 120, JT], BF16, kind="ExternalInput").ap()
    wl0_d = nc.dram_tensor("wl0", [120, 4 * 121], BF16, kind="ExternalInput").ap()
    wmid_d = nc.dram_tensor("wmid", [121, NMID * 121], BF16, kind="ExternalInput").ap()
    wl15_d = nc.dram_tensor("wl15", [121, 120], BF16, kind="ExternalInput").ap()
    wbias_d = nc.dram_tensor("wbias", [121, 15], F32, kind="ExternalInput").ap()
    out_d = nc.dram_tensor("out", [NT, 128, 960], BF16, kind="ExternalOutput").ap()

    with tile.TileContext(nc) as tc, ExitStack() as ctx:
        sc = ctx.enter_context(tc.tile_pool(name="sc", bufs=1))
        sx = ctx.enter_context(tc.tile_pool(name="sx", bufs=8))
        sh = ctx.enter_context(tc.tile_pool(name="sh", bufs=2))
        sv = ctx.enter_context(tc.tile_pool(name="sv", bufs=4))
        sout = ctx.enter_context(tc.tile_pool(name="sout", bufs=2))
        pmA = ctx.enter_context(tc.tile_pool(name="pmA", bufs=2, space="PSUM"))
        pmD = ctx.enter_context(tc.tile_pool(name="pmD", bufs=1, space="PSUM"))
        pout = ctx.enter_context(tc.tile_pool(name="pout", bufs=2, space="PSUM"))

        consts = {}

        def _load_consts(names):
            specs = {
                "wl0": (wl0_d, [120, 4 * 121], BF16),
                "wmid": (wmid_d, [121, NMID * 121], BF16),
                "wl15": (wl15_d, [121, 120], BF16),
                "wbias": (wbias_d, [121, 15], F32),
            }
            for name in names:
                dram, shape, dt = specs[name]
                t = sc.tile(shape, dt, name=f"c_{name}", tag=name)
                nc.sync.dma_start(t[:], dram)
                consts[name] = t

        def bias_ap(l):
            return consts["wbias"][:, l:l + 1]

        def act_full(dst, psum, l):
            nc.scalar.activation(dst, psum, AF.Prelu, bias=bias_ap(l),
                                 scale=1.0, alpha=0.5)

        def act_dve_full(dst, psum, l):
            v = sv.tile([121, JT], BF16, tag="v")
            nc.vector.tensor_scalar(v[:], psum, bias_ap(l), None, ALU.add)
            nc.vector.scalar_tensor_tensor(dst, v[:], 0.5, v[:],
                                           ALU.mult, ALU.max)

        dve_pending = []

        def flush_dve(keep=0):
            while len(dve_pending) > keep:
                dve_pending.pop(0)()

        def act_hyb_half(dst, psum, l):
            """DVE computes v from PSUM, Pool scales, DVE finishes with a 2x
            bf16 max. The final op is deferred (dve_pending) so it does not
            head-of-line block the DVE queue while Pool works."""
            v = sv.tile([121, 512], BF16, tag="v")
            u = sv.tile([121, 512], BF16, tag="u")
            nc.vector.tensor_scalar(v[:], psum, bias_ap(l), None, ALU.add)
            nc.gpsimd.tensor_scalar(u[:], v[:], 0.5, None, ALU.mult)

            def fin():
                nc.vector.tensor_tensor(dst, v[:], u[:], ALU.max)
            dve_pending.append(fin)
            flush_dve(keep=1)

        loop_ctx = tc.For_i(0, reps, 1) if reps > 1 else None
        if loop_ctx is not None:
            ctx.enter_context(loop_ctx)

        # DMA order: L0 consts, first x blocks, remaining consts, rest of x
        _load_consts(["wl0", "wbias"])
        xblk = {}
        for k in range(NBLK):
            xblk[k] = sx.tile([120, JT], BF16, name=f"x{k}", tag="x")
            nc.sync.dma_start(xblk[k][:], x_d[k])
            if k == 7:
                _load_consts(["wmid", "wl15"])

        def stage_l0_chunks(t):
            """L0 for tile t as a list of closures (small PE bursts)."""
            blocks = [4 * t + k for k in range(4)] if t < 10 else [40, 41, 42]
            h[t] = sh.tile([121, JT], BF16, name=f"h{t}", tag=f"h{t}")
            eng = ENGINE_TABLE[("act", 0, t)]
            chunks = []
            state = {}
            n = len(blocks)

            if eng == "act":
                def mk(i0, i1, first, last):
                    def emit():
                        if first:
                            state["p"] = pmA.tile([121, JT], F32,
                                                  name=f"p0_{t}", tag="p")
                        p = state["p"]
                        for i in range(i0, i1):
                            w = consts["wl0"][:, 121 * i:121 * i + 121]
                            for half in range(2):
                                nc.tensor.matmul(
                                    p[:, 512 * half:512 * half + 512],
                                    w,
                                    xblk[blocks[i]][:, 512 * half:512 * half + 512],
                                    start=(i == 0), stop=(i == n - 1),
                                )
                        if last:
                            act_full(h[t][:], p[:], 0)
                    return emit
                chunks.append(mk(0, 2, True, False))
                chunks.append(mk(2, n, False, True))
            else:
                def mk(i0, i1, first, last):
                    def emit():
                        if first:
                            state["p"] = pmD.tile([121, JT], F32,
                                                  name=f"p0_{t}", tag="p")
                        p = state["p"]
                        for i in range(i0, i1):
                            w = consts["wl0"][:, 121 * i:121 * i + 121]
                            for half in range(2):
                                nc.tensor.matmul(
                                    p[:, 512 * half:512 * half + 512],
                                    w,
                                    xblk[blocks[i]][:, 512 * half:512 * half + 512],
                                    start=(i == 0), stop=(i == n - 1),
                                )
                        if last:
                            act_dve_full(h[t][:], p[:], 0)
                    return emit
                chunks.append(mk(0, 2, True, False))
                chunks.append(mk(2, n, False, True))
            return chunks

        def stage_mid(l, t):
            w = consts["wmid"][:, 121 * (l - 1):121 * (l - 1) + 121]
            hn = sh.tile([121, JT], BF16, name=f"h{l}_{t}", tag=f"h{t}")
            if ENGINE_TABLE[("act", l, t)] == "act":
                p = pmA.tile([121, JT], F32, name=f"pm{l}_{t}", tag="p")
                for half in range(2):
                    nc.tensor.matmul(
                        p[:, 512 * half:512 * half + 512],
                        w,
                        h[t][:, 512 * half:512 * half + 512],
                        start=True, stop=True,
                    )
                act_full(hn[:], p[:], l)
            else:
                p = pmD.tile([121, JT], F32, name=f"pm{l}_{t}", tag="p")
                for half in range(2):
                    nc.tensor.matmul(
                        p[:, 512 * half:512 * half + 512],
                        w,
                        h[t][:, 512 * half:512 * half + 512],
                        start=True, stop=True,
                    )
                act_dve_full(hn[:], p[:], l)
            h[t] = hn

        def stage_l15(t):
            s_o = sout.tile([128, 960], BF16, name=f"so{t}", tag="so")
            for half in range(2):
                po = pout.tile([128, 480], F32, name=f"po{t}_{half}", tag="po")
                for b4 in range(4):
                    b = 4 * half + b4
                    nc.tensor.matmul(
                        po[:, 120 * b4:120 * b4 + 120],
                        h[t][:, 128 * b:128 * b + 128],
                        consts["wl15"][:],
                        start=True, stop=True,
                    )
                dst = s_o[:, 480 * half:480 * half + 480]
                ceng = ENGINE_TABLE[("copy", t, half)]
                if ceng == "act":
                    nc.scalar.activation(dst, po[:], AF.Copy)
                else:
                    nc.vector.tensor_copy(dst, po[:])
            nc.sync.dma_start(out_d[t], s_o[:])

        # Skewed wavefront: tile t runs layer s - t at step s, so L0 / mid /
        # L15 stages of different tiles interleave and all engines stay fed.
        h = {}
        def _start(t):
            return t // 2
        for s in range(_start(NT - 1) + 16):
            flush_dve(keep=0)
            l0c, mids, l15s = [], [], []
            for t in range(NT):
                l = s - _start(t)
                if l == 0:
                    l0c.extend(stage_l0_chunks(t))
                elif 1 <= l <= NMID:
                    mids.append((l, t))
                elif l == NMID + 1:
                    l15s.append(t)
            for t in l15s:
                stage_l15(t)
            for (l, t) in mids:
                stage_mid(l, t)
            for c in l0c:
                c()
        flush_dve(keep=0)

    if split_waits:
        _split_multi_waits(nc)
    return nc


_NC_CACHE = {}


def kernel(x, W_in, b_in, W_mid, b_mid, W_out, b_out):
    x = np.asarray(x, np.float32)
    W_in = np.asarray(W_in, np.float32)
    b_in = np.asarray(b_in, np.float32)
    W_mid = np.asarray(W_mid, np.float32)
    b_mid = np.asarray(b_mid, np.float32)
    W_out = np.asarray(W_out, np.float32)
    b_out = np.asarray(b_out, np.float32)

    if "nc" not in _NC_CACHE:
        _NC_CACHE["nc"] = _build_nc()
    nc = _NC_CACHE["nc"]

    consts = _pack_weights(W_in, b_in, W_mid, b_mid, W_out, b_out)

    in_maps = []
    for c in range(NCORES):
        xc = _pack_x_core(x[c * BC:(c + 1) * BC])
        in_maps.append({"x": xc, **consts})

    res = run_bass_kernel_spmd(nc, in_maps, list(range(NCORES)))

    outs = []
    for c in range(NCORES):
        oc = np.asarray(res.results[c]["out"], np.float32)  # [NT, 128, 960]
        oc = oc.reshape(NT, 128, 2, 4, 12, HID).transpose(0, 4, 2, 3, 1, 5)
        outs.append(oc.reshape(NT * 12 * JT, HID)[:BC])
    return np.ascontiguousarray(np.concatenate(outs, axis=0).astype(np.float32))

